# revision 17
# baseline (speedup 1.0000x reference)
"""CrossAttentionFusion kernel for 8x Trainium2 NeuronCores.

Sharding: data-parallel over batch B=8 -> one batch element per core.
No collectives needed; weights replicated to all cores.

Algebraically folded formulation (host-side weight folding):
  - K-fold: logits = Zq (Wq^T Wk) Zkv^T, so the two K projections fold
    into the Q-side weight M_d = Wq^T Wk, and the raw Zkv^T (already in
    SBUF layout for the S^T matmul) serves as the keys. The Q-side
    logit bias c_d = bq @ Wk survives as the Q' bias; the K-side bias
    is softmax-invariant and dropped.
  - V-fold: attn @ (Zkv Wv^T) @ Wf^T = (attn @ Zkv) @ (Wf Wv)^T, so the
    two V projections fold into per-direction final weights
    F_d = Wf @ Wv. Raw Zkv (natural layout) serves as the values; the
    final projection becomes a 12-matmul PSUM chain (6 per direction)
    instead of 6. Net PE work: 7 SDD + 4 SSD MACs -> 4 SDD + 4 SSD.
  - V biases fold into the final bias (softmax rows sum to 1):
    bf_eff = bf + Wf (bvl + bvg).

fp8-e4m3 DoubleRow matmuls for the attention core (S = K^T.Q and
U = V.E); the fp8 K (= Z^T) and V (= Z) tiles are pre-quantized on the
host at scale 4 and DMAd directly (no on-chip eviction work). Numerics
(same scales as the pre-fold baseline):
  - Q'^T evicted to fp8 at scale 4 (bias c_d pre-scaled by 4).
  - S psum = 16*logits; exp on ScalarE with scale (1/sqrt(D))/16 and
    bias ln(4) -> etmp = 4*exp(l) bf16; the VectorE then subtracts 4C
    (C=1.0625 ~ E[exp(l)]) and stores delta = 4*(exp(l)-C) in fp8.
  - U psum = sum delta*Z8 = 16*(N - C*zcQ); eviction adds back the
    per-partition bias 2*C*zcQ[d] (host-computed column sums of the
    quantized Z) at scale 1/8 -> usb = 2*N bf16, unnormalized.
  - softmax denominator on the PE: one DoubleRow ones-matmul per k-pair
    accumulates r_ps = sum_k delta; a tiny DVE affine recovers
    2r = 0.5*r_ps + 4096*C, then reciprocal + K=1 f32r broadcast
    matmul; zm = usb * (1/(2r)) = N/r.
  - The constant-E-channel part of the Z-quantization error folds into
    the final bias host-side: F_d @ (zcT-zcQ)/2048 per direction.

All large inputs are host-packed partition-major ([128, ...] with the
SBUF per-partition layout precomputed) so each logical load is ONE
large DMA: the per-transfer ~2us completion latency serializes within
a DMA ring, capping a ring at ~80GB/s for 128KB chunks but ~230GB/s at
1MB. The three rings (sync/scalar HWDGE, gpsimd SWDGE) carry disjoint
critical chains: sync = biases+M0+K-tiles, scalar = Z^T q-block
chunks, gpsimd = V-tiles.

Per-core structure: warmup matmuls bridge the HAM clock-gate ramp and
the initial DMA (~15us); per direction, all four q-blocks' Q'
projections run upfront (DMA runway for the fp8 K/V tiles), then
attention per q-block of 512 with U in two 3-bank PSUM passes. dir0's
normalized N0^T goes to DRAM scratch (bf16, one packed DMA per
q-block); dir1 loads it back (one DMA per q-block) and the final
projection (one q-block behind) accumulates both directions'
contributions in one 12-matmul PSUM chain. The bf16 Z^T buffer is
shared between directions (dir1's Z_lstm^T overwrites after dir0's Q'
projections finish reading).

PSUM split: 4 banks rotate the S/Q'-proj/final psums, 3 banks for the
two U passes, 1 for the denominator.

Numpy bit-sim of the folded quantization chain predicts rel err
1.41e-2 vs the fp32 reference (threshold 2e-2); measured 1.40e-2.
"""

import numpy as np
import ml_dtypes

import concourse.bass as bass
import concourse.mybir as mybir
import concourse.tile as tile
from concourse import bacc
from concourse.bass_utils import run_bass_kernel_spmd

S = 2048
D = 768
P = 128
NDC = D // P      # 6 chunks of the model dim
NPR = NDC // 2    # 3 DoubleRow pairs of the model dim
NSC = S // P      # 16 chunks of the sequence
NSP = NSC // 2    # 8 DoubleRow pairs of the sequence
QB = 512          # q-block width
NQB = S // QB     # 4 q-blocks
NH = 2            # halves of D for N=384 matmuls
HWID = D // NH    # 384
NCORES = 8
INV_SQRT_D = float(1.0 / np.sqrt(D))

S_KQ = 4.0                           # scale of stored K^T / Q'^T fp8
SC_EXP = INV_SQRT_D / (S_KQ * S_KQ)  # exp input scale (psum = 16*logits)
S_E = 4.0                            # scale of stored delta fp8
C_E = 1.0625                         # offset: delta = S_E*(exp(l) - C_E)
EXP_BIAS = float(np.log(S_E))        # etmp = exp(l + ln4) = 4*exp(l)
S_V = 4.0                            # scale of stored Z-values fp8
SC_USB = 2.0 / (S_E * S_V)           # U psum -> usb units (2*N)

F32 = mybir.dt.float32
F32R = mybir.dt.float32r
BF16 = mybir.dt.bfloat16
F8 = mybir.dt.float8e4
DR = mybir.MatmulPerfMode.DoubleRow

# (kv_src, q_src) per direction; sources index (graph, lstm)
DIRS = [
    (1, 0),   # graph queries attend lstm keys/values
    (0, 1),   # lstm queries attend graph keys/values
]

AF = mybir.ActivationFunctionType


def build_kernel_body(nc, tc, ztp_dram, z8tp_dram, z8np_dram, mt, ft, bp,
                      bv, br, out_ap):
    import contextlib
    with contextlib.ExitStack() as stk:
        persist = stk.enter_context(tc.tile_pool(name="persist", bufs=1))
        psum = stk.enter_context(tc.tile_pool(name="psum", bufs=1, space="PSUM"))
        work = stk.enter_context(tc.tile_pool(name="work", bufs=1))
        dram = stk.enter_context(tc.tile_pool(name="dram", bufs=1, space="DRAM"))

        # ---- warmup input first: the PE's first matmul waits only on this
        wu = work.tile([P, QB], BF16, name="wu", tag="wu", bufs=1)
        nc.vector.memset(wu[:], 0.0)

        # ---- constants ----
        ones_col_f = persist.tile([P, 1], F32, name="ones_col_f", tag="ones_col_f")
        nc.vector.memset(ones_col_f[:], 1.0)
        ones_row_f = persist.tile([1, P], F32, name="ones_row_f", tag="ones_row_f")
        nc.vector.memset(ones_row_f[:], 1.0)
        ones2_f = persist.tile([P, 32], F32, name="ones2_f", tag="ones2_f")
        nc.vector.memset(ones2_f[:], 1.0)
        ln4_bias = persist.tile([P, 1], F32, name="ln4_bias", tag="ln4_bias")
        nc.vector.memset(ln4_bias[:], EXP_BIAS)
        ones_row_r = persist.tile([1, P], F32R, name="ones_row_r", tag="ones_row_r")
        with nc.allow_low_precision(reason="f32r ones (exact)"):
            nc.vector.tensor_copy(ones_row_r[:], ones_row_f[:])
        # fp8 ones pair-column for the PE row-sum of delta; padded so the
        # DoubleRow weight AP's pair-dim step is 16B-aligned
        ones8_t = persist.tile([P, 2, 16], F8, name="ones8_t", tag="ones8_t")
        nc.scalar.activation(ones8_t[:, :, :], ones2_f[:], AF.Copy)
        ones8 = ones8_t[:, :, 0:1]

        # ---- PE warmup asap (HAM clock-gate), before any DMA deps ----
        # pure filler until the Q'-proj-critical DMA lands (~15us); the
        # HAM warm-up latency is run-variable (4-18us), so back-to-back
        # filler beats starting stall-broken real work early.
        for i in range(28):
            wps = psum.tile([P, QB], F32, name=f"wps{i}", tag="S", bufs=4)
            nc.tensor.matmul(wps[:], lhsT=wu[:, 0:P], rhs=wu[:],
                             start=True, stop=True)

        # ---- small parameter tensors ----
        # Q' biases on the sync ring ahead of M0 (needed by the first Q'
        # evictions); bv/br ahead of the gpsimd ring's V tiles.
        bp_sb = []
        for di in range(2):
            t = persist.tile([P, NDC], F32, name=f"bp_{di}", tag=f"bp_{di}")
            nc.sync.dma_start(out=t[:], in_=bp[di][:, :])
            bp_sb.append(t)
        bv_sb = [persist.tile([P, NDC], F32, name=f"bv_{di}", tag=f"bv_{di}")
                 for di in range(2)]
        br_sb = persist.tile([1, D], F32, name="br_Wf", tag="br_Wf")
        bias_bc = persist.tile([P, D], F32, name="bias_bc", tag="bias_bc")

        # dir0's folded Q weight M0 (one 1.15MB transfer, sync ring)
        m_all = [work.tile([P, NDC, D], BF16, name=f"m{di}", tag=f"m{di}",
                           bufs=1) for di in range(2)]
        nc.sync.dma_start(out=m_all[0][:, :, :], in_=mt[0][:, :, :])

        # Z^T packed [p, dc, s], shared between directions: dir0 loads
        # Z_graph^T (its Q' source) in q-block column chunks (768KB each,
        # scalar ring); dir1 overwrites with Z_lstm^T later.
        zt = persist.tile([P, NDC, S], BF16, name="zt", tag="zt")
        for sb in range(NQB):
            nc.scalar.dma_start(
                out=zt[:, :, sb * QB:(sb + 1) * QB],
                in_=ztp_dram[0][:, :, sb * QB:(sb + 1) * QB])

        # usb bias (needed ~60us) + final bias (needed ~35us)
        for di in range(2):
            nc.gpsimd.dma_start(out=bv_sb[di][:], in_=bv[di][:, :])
        nc.gpsimd.dma_start(out=br_sb[:], in_=br[:, :])

        # fp8 K tiles (= Z_kv^T at scale 4) per direction, host-packed
        # [p, d-chunk, s]; dir0's (lstm) in one 1.5MB transfer on sync,
        # needed at S of qb0 (~46us).
        kt = [persist.tile([P, NDC, S], F8, name=f"kt{di}", tag=f"kt{di}")
              for di in range(2)]
        nc.sync.dma_start(out=kt[0][:, :, :],
                          in_=z8tp_dram[DIRS[0][0]][:, :, :])
        # fp8 V tiles (= Z_kv at scale 4) host-packed [p, s-chunk, d];
        # 2-pair groups (384KB) on gpsimd, needed in kp order from ~48us.
        v_all = [persist.tile([P, NSC, D], F8, name=f"v{di}", tag=f"v{di}")
                 for di in range(2)]
        for g in range(4):
            nc.gpsimd.dma_start(
                out=v_all[0][:, g * 4:(g + 1) * 4, :],
                in_=z8np_dram[DIRS[0][0]][:, g * 4:(g + 1) * 4, :])

        # final weights [p, fdi*6+dc, e], one transfer per direction
        # (needed from dir1 qb1, ~210us)
        wf_all = persist.tile([P, 2 * NDC, D], BF16, name="wf", tag="wf")

        # DRAM scratch holding dir0's normalized N0^T, packed [p, dc, s]
        zfg_dram = dram.tile([P, NDC, S], BF16, name="zfg_scratch", tag="zfg")

        # ---- the two attention directions ----
        for di, (kv_src, q_src) in enumerate(DIRS):
            with tc.tile_pool(name=f"dir{di}", bufs=1) as dp:
                if di == 0:
                    # fp32 broadcast of the (folded) final bias across
                    # partitions; deferred here so the two PE matmuls
                    # don't sit between warmup and the first Q' matmul
                    for h in range(NH):
                        bps = psum.tile([P, HWID], F32, name=f"bps{h}",
                                        tag="S", bufs=4)
                        nc.tensor.matmul(bps[:], lhsT=ones_row_f[:],
                                         rhs=br_sb[0:1, h * HWID:(h + 1) * HWID],
                                         start=True, stop=True)
                        nc.vector.tensor_copy(
                            bias_bc[:, h * HWID:(h + 1) * HWID], bps[:])
                    # dir1 prefetches, behind dir0's critical loads:
                    # M1 + final weights + dir1 K tiles on sync; the
                    # Z_lstm^T overwrite too (WAR clears once the Q'
                    # projections below finish reading zt); V on gpsimd.
                    nc.sync.dma_start(out=m_all[1][:, :, :],
                                      in_=mt[1][:, :, :])
                    for fdi in range(2):
                        nc.sync.dma_start(
                            out=wf_all[:, fdi * NDC:(fdi + 1) * NDC, :],
                            in_=ft[fdi][:, :, :])
                    nc.sync.dma_start(out=kt[1][:, :, :],
                                      in_=z8tp_dram[DIRS[1][0]][:, :, :])
                    for g in range(4):
                        nc.gpsimd.dma_start(
                            out=v_all[1][:, g * 4:(g + 1) * 4, :],
                            in_=z8np_dram[DIRS[1][0]][:, g * 4:(g + 1) * 4, :])

                # ---- Q'^T for all four q-blocks upfront (DMA runway) ----
                qt = [[dp.tile([P, 2, QB], F8, name=f"qt{di}_{qb}_{j}",
                               tag=f"qt_{qb}_{j}") for j in range(NPR)]
                      for qb in range(NQB)]
                for qb in range(NQB):
                    for ec in range(NDC):
                        ps = psum.tile([P, QB], F32, name=f"ps_q{di}_{qb}_{ec}",
                                       tag="S", bufs=4)
                        for dc in range(NDC):
                            nc.tensor.matmul(
                                ps[:],
                                lhsT=m_all[di][:, dc, ec * P:(ec + 1) * P],
                                rhs=zt[:, dc, qb * QB:(qb + 1) * QB],
                                start=(dc == 0), stop=(dc == NDC - 1))
                        nc.scalar.activation(
                            qt[qb][ec // 2][:, ec % 2:ec % 2 + 1, :], ps[:],
                            AF.Identity, bias=bp_sb[di][:, ec:ec + 1],
                            scale=S_KQ)

                if di == 0:
                    # Z_lstm^T overwrite for dir1's Q' projections: MUST be
                    # emitted after the Q' matmuls above so the WAR
                    # dependency on dir0's reads of zt is tracked.
                    for sb in range(NQB):
                        nc.sync.dma_start(
                            out=zt[:, :, sb * QB:(sb + 1) * QB],
                            in_=ztp_dram[1][:, :, sb * QB:(sb + 1) * QB])

                # ---- attention, one q-block at a time ----
                # final projection (dir1) runs one q-block behind; pend
                # holds the previous block's normalized N^T of both dirs.
                pend = None

                def final_proj(n0qb, n1qb, qb):
                    for i in range(QB // P):
                        ostage = work.tile([P, D], F32, name=f"os{qb}_{i}",
                                           tag="ostage", bufs=2)
                        row0 = qb * QB + i * P
                        for h in range(NH):
                            fp = psum.tile([P, HWID], F32, name=f"fp{qb}_{i}_{h}",
                                           tag="S", bufs=4)
                            for fdi in range(2):
                                for dc in range(NDC):
                                    lhsT = (n0qb[:, dc, i * P:(i + 1) * P]
                                            if fdi == 0 else
                                            n1qb[dc][:, i * P:(i + 1) * P])
                                    nc.tensor.matmul(
                                        fp[:], lhsT=lhsT,
                                        rhs=wf_all[:, fdi * NDC + dc,
                                                   h * HWID:(h + 1) * HWID],
                                        start=(fdi == 0 and dc == 0),
                                        stop=(fdi == 1 and dc == NDC - 1))
                            nc.vector.tensor_add(
                                ostage[:, h * HWID:(h + 1) * HWID], fp[:],
                                bias_bc[:, h * HWID:(h + 1) * HWID])
                        # one packed [128, 768] output DMA per row-chunk
                        nc.sync.dma_start(out=out_ap[row0:row0 + P, :],
                                          in_=ostage[:, :])

                for qb in range(NQB):
                    if di == 1:
                        zfg_in = work.tile([P, NDC, QB], BF16,
                                           name=f"zfi{qb}", tag="zfg_in",
                                           bufs=2)
                        nc.sync.dma_start(
                            out=zfg_in[:, :, :],
                            in_=zfg_dram[:, :, qb * QB:(qb + 1) * QB])

                    # previous q-block's final projection (PE-dense filler
                    # while this block's S-phase evictions run on ACT/DVE)
                    if pend is not None:
                        final_proj(*pend)
                        pend = None

                    # S^T chunks (DoubleRow) -> exp -> delta fp8 -> U pass 1.
                    # The softmax denominator accumulates on the PE via one
                    # DoubleRow ones-matmul per k-pair (r_ps = sum_k delta).
                    u_ps = [psum.tile([P, QB], F32, name=f"u{qb}_{dc}",
                                      tag="pu", bufs=3) for dc in range(3)]
                    e_pairs = [work.tile([P, 2, QB], F8, name=f"et{qb}_{kp}",
                                         tag="et", bufs=10)
                               for kp in range(NSP)]
                    r_ps = psum.tile([1, QB], F32, name=f"r{qb}", tag="r", bufs=1)

                    def u1_round(kp):
                        for dc in range(3):
                            nc.tensor.matmul(
                                u_ps[dc][:],
                                lhsT=v_all[di][:, 2 * kp:2 * kp + 2,
                                               dc * P:(dc + 1) * P],
                                rhs=e_pairs[kp][:, :, :],
                                start=(kp == 0), stop=(kp == NSP - 1),
                                perf_mode=DR)
                        nc.tensor.matmul(
                            r_ps[0:1, :], lhsT=ones8,
                            rhs=e_pairs[kp][:, :, :],
                            start=(kp == 0), stop=(kp == NSP - 1),
                            perf_mode=DR)

                    # U pass 1 runs one k-pair behind the S matmuls so the
                    # exp->subtract eviction chain of a pair hides under the
                    # NEXT pair's S matmuls instead of stalling the in-order
                    # PE queue.
                    pend_kp = None
                    for kc in range(NSC):
                        sp = psum.tile([P, QB], F32, name=f"s{qb}_{kc}",
                                       tag="S", bufs=4)
                        for j in range(NPR):
                            nc.tensor.matmul(
                                sp[:],
                                lhsT=kt[di][:, 2 * j:2 * j + 2,
                                            kc * P:(kc + 1) * P],
                                rhs=qt[qb][j][:, :, :],
                                start=(j == 0), stop=(j == NPR - 1),
                                perf_mode=DR)
                        etmp = work.tile([P, QB], BF16, name=f"etm{qb}_{kc}",
                                         tag="etmp", bufs=3)
                        nc.scalar.activation(etmp[:], sp[:], AF.Exp,
                                             bias=ln4_bias[:, 0:1],
                                             scale=SC_EXP)
                        eslice = e_pairs[kc // 2][:, kc % 2:kc % 2 + 1, :]
                        nc.vector.tensor_scalar_add(eslice, etmp[:],
                                                    -S_E * C_E)
                        if kc % 2 == 1:
                            if pend_kp is not None:
                                u1_round(pend_kp)
                            pend_kp = kc // 2
                    u1_round(pend_kp)

                    # unnormalized evictions of pass 1 (frees pu banks fast);
                    # bias restores the C*zcQ channel: usb = 2*N
                    usb = [None] * NDC
                    for dc in range(NDC):
                        usb[dc] = work.tile([P, QB], BF16, name=f"usb{qb}_{dc}",
                                            tag="usb", bufs=6)
                    for dc in range(3):
                        nc.scalar.activation(usb[dc][:], u_ps[dc][:],
                                             AF.Identity,
                                             bias=bv_sb[di][:, dc:dc + 1],
                                             scale=SC_USB)

                    # U pass 2; the denominator affine + reciprocal run on the
                    # DVE as soon as U pass 1 (and with it r_ps) completes,
                    # hiding their latency under the U2 matmuls.
                    rr = work.tile([1, QB], F32, name=f"rr{qb}", tag="rr", bufs=1)
                    rsb = work.tile([1, QB], F32R, name=f"rsb{qb}", tag="rsb",
                                    bufs=1)
                    u_ps2 = [psum.tile([P, QB], F32, name=f"u2{qb}_{dc}",
                                       tag="pu", bufs=3) for dc in range(3)]
                    for kp in range(NSP):
                        for i, dc in enumerate(range(3, NDC)):
                            nc.tensor.matmul(
                                u_ps2[i][:],
                                lhsT=v_all[di][:, 2 * kp:2 * kp + 2,
                                               dc * P:(dc + 1) * P],
                                rhs=e_pairs[kp][:, :, :],
                                start=(kp == 0), stop=(kp == NSP - 1),
                                perf_mode=DR)
                        if kp == 0:
                            # 2r = 0.5*r_ps + 2048*S_E*C_E/2, then reciprocal
                            nc.vector.tensor_scalar(
                                rr[:], r_ps[0:1, :], 0.5,
                                float(S / 2 * S_E * C_E),
                                mybir.AluOpType.mult, mybir.AluOpType.add)
                            with nc.allow_low_precision(
                                    reason="f32r reciprocal"):
                                nc.vector.reciprocal(rsb[:], rr[:])
                    for i, dc in enumerate(range(3, NDC)):
                        nc.scalar.activation(usb[dc][:], u_ps2[i][:],
                                             AF.Identity,
                                             bias=bv_sb[di][:, dc:dc + 1],
                                             scale=SC_USB)

                    # broadcast 1/(2r) across partitions (reciprocal done)
                    rb_ps = psum.tile([P, QB], F32, name=f"rb{qb}", tag="r", bufs=1)
                    nc.tensor.matmul(rb_ps[:], lhsT=ones_row_r[:], rhs=rsb[:],
                                     start=True, stop=True)
                    rb_sb = work.tile([P, QB], F32, name=f"rbs{qb}", tag="rb_sb",
                                      bufs=1)
                    nc.vector.tensor_copy(rb_sb[:], rb_ps[:])

                    # normalize; dir0 -> DRAM scratch (one packed DMA),
                    # dir1 -> pend for the combined final projection (the
                    # cross-direction sum happens in the final PSUM chain)
                    if di == 0:
                        zst = work.tile([P, NDC, QB], BF16, name=f"zst{qb}",
                                        tag="zst", bufs=2)
                        for dc in range(NDC):
                            nc.vector.tensor_mul(zst[:, dc, :], usb[dc][:],
                                                 rb_sb[:])
                        nc.sync.dma_start(
                            out=zfg_dram[:, :, qb * QB:(qb + 1) * QB],
                            in_=zst[:, :, :])
                    else:
                        zmqb = [None] * NDC
                        for dc in range(NDC):
                            zm = work.tile([P, QB], BF16, name=f"zm{qb}_{dc}",
                                           tag="zm", bufs=8)
                            nc.vector.tensor_mul(zm[:], usb[dc][:], rb_sb[:])
                            zmqb[dc] = zm
                        pend = (zfg_in, zmqb, qb)

                if pend is not None:
                    final_proj(*pend)
                    pend = None


_CACHED = {}


def _build_nc():
    if "nc" in _CACHED:
        return _CACHED["nc"]
    nc = bacc.Bacc("TRN2", target_bir_lowering=False, debug=False)
    # all large inputs host-packed partition-major (see make_in_maps)
    ztp = [nc.dram_tensor(f"ztp_{n}", [P, NDC, S], BF16,
                          kind="ExternalInput") for n in ("graph", "lstm")]
    z8tp = [nc.dram_tensor(f"z8tp_{n}", [P, NDC, S], F8,
                           kind="ExternalInput") for n in ("graph", "lstm")]
    z8np = [nc.dram_tensor(f"z8np_{n}", [P, NSC, D], F8,
                           kind="ExternalInput") for n in ("graph", "lstm")]
    mt = [nc.dram_tensor(f"mt_{di}", [P, NDC, D], BF16, kind="ExternalInput")
          for di in range(2)]
    ft = [nc.dram_tensor(f"ft_{di}", [P, NDC, D], BF16, kind="ExternalInput")
          for di in range(2)]
    bp = [nc.dram_tensor(f"bp_{di}", [P, NDC], F32, kind="ExternalInput")
          for di in range(2)]
    bv = [nc.dram_tensor(f"bv_{di}", [P, NDC], F32, kind="ExternalInput")
          for di in range(2)]
    br = nc.dram_tensor("br_Wf", [1, D], F32, kind="ExternalInput")
    out = nc.dram_tensor("out", [S, D], F32, kind="ExternalOutput")

    with tile.TileContext(nc) as tc:
        build_kernel_body(
            nc, tc,
            [v.ap() for v in ztp],
            [v.ap() for v in z8tp],
            [v.ap() for v in z8np],
            [v.ap() for v in mt],
            [v.ap() for v in ft],
            [v.ap() for v in bp],
            [v.ap() for v in bv],
            br.ap(),
            out.ap(),
        )
    nc.compile()
    _CACHED["nc"] = nc
    return nc


def _pack(x, nchunk):
    """[nchunk*P, cols...] -> [P, nchunk, cols...] partition-major."""
    return np.ascontiguousarray(
        x.reshape(nchunk, P, *x.shape[1:]).swapaxes(0, 1))


def make_in_maps(inputs):
    """Host-side sharding: one batch element per core; weights replicated.
    Folds Wq^T Wk -> M_d (+ logit bias c_d = bq Wk), Wf Wv -> F_d, V and
    K biases into c_d / the final bias, quantizes Z to fp8 at scale 4 in
    both layouts (keys Z^T, values Z), computes the per-core column sums
    of the quantized Z (usb bias + final-bias correction), and packs
    every large tensor partition-major for single-DMA loads."""
    bf16 = ml_dtypes.bfloat16
    f8 = ml_dtypes.float8_e4m3
    zg = np.asarray(inputs["Z_graph"], dtype=np.float32)
    zl = np.asarray(inputs["Z_lstm"], dtype=np.float32)
    W = {n: np.asarray(inputs[n], dtype=np.float64)
         for n in ("Wqg", "Wkl", "Wvl", "Wql", "Wkg", "Wvg", "Wf")}
    b = {n: np.asarray(inputs[n], dtype=np.float64)
         for n in ("bqg", "bkl", "bvl", "bql", "bkg", "bvg", "bf")}

    shared = {}
    # folded Q-side weights/biases: dir0 q=graph k=lstm, dir1 q=lstm k=graph
    M = [W["Wqg"].T @ W["Wkl"], W["Wql"].T @ W["Wkg"]]
    c = [b["bqg"] @ W["Wkl"], b["bql"] @ W["Wkg"]]
    # folded final weights; stored transposed ([d_in, d_out] layout)
    F = [W["Wf"] @ W["Wvl"], W["Wf"] @ W["Wvg"]]
    for di in range(2):
        shared[f"mt_{di}"] = _pack(M[di].astype(bf16), NDC)
        shared[f"ft_{di}"] = _pack(
            np.ascontiguousarray(F[di].T).astype(bf16), NDC)
        shared[f"bp_{di}"] = np.ascontiguousarray(
            (c[di] * S_KQ).astype(np.float32).reshape(NDC, P).T)
    # K bias is softmax-invariant -> dropped. V biases pass through
    # attention unchanged (softmax rows sum to 1) -> final bias.
    bf_eff = b["bf"] + W["Wf"] @ (b["bvl"] + b["bvg"])

    in_maps = []
    for cr in range(NCORES):
        m = dict(shared)
        zsrc = [zg[cr], zl[cr]]                       # [S, D] fp32 each
        names = ["graph", "lstm"]
        zcQ = [None, None]
        dzc = [None, None]
        for si in range(2):
            z = zsrc[si]
            m[f"ztp_{names[si]}"] = _pack(
                np.ascontiguousarray(z.T).astype(bf16), NDC)
            z8n = (z * S_V).astype(f8)                # values, natural
            m[f"z8np_{names[si]}"] = _pack(z8n, NSC)
            m[f"z8tp_{names[si]}"] = _pack(np.ascontiguousarray(z8n.T), NDC)
            zq = z8n.astype(np.float32) / S_V
            zcQ[si] = zq.sum(axis=0, dtype=np.float64)
            dzc[si] = z.astype(np.float64).sum(axis=0) - zcQ[si]
        for di, (kv_src, _) in enumerate(DIRS):
            m[f"bv_{di}"] = np.ascontiguousarray(
                (2.0 * C_E * zcQ[kv_src]).astype(np.float32).reshape(NDC, P).T)
        # constant-E-channel part of the Z-quantization error, folded
        # through the per-direction final weights into the final bias
        bfc = (bf_eff
               + F[0] @ (dzc[DIRS[0][0]] / S)
               + F[1] @ (dzc[DIRS[1][0]] / S))
        m["br_Wf"] = np.ascontiguousarray(
            bfc.astype(np.float32).reshape(1, D))
        in_maps.append(m)
    return in_maps


def run(inputs, trace=False, **kwargs):
    nc = _build_nc()
    in_maps = make_in_maps(inputs)
    res = run_bass_kernel_spmd(nc, in_maps, list(range(NCORES)),
                               trace=trace, **kwargs)
    out = np.stack([res.results[c]["out"] for c in range(NCORES)], axis=0)
    return out.astype(np.float32), res


def kernel(**inputs):
    out, _ = run(inputs, trace=False)
    return out
